# revision 19
# baseline (speedup 1.0000x reference)
"""Trainium2 Bass kernel: 3-layer actor MLP over [B=256, K=1000] actions.

Math (per reference):
    h1 = relu(af @ W1_a + state @ W1_s + b1)   # [B,K,256]
    h2 = relu(h1 @ W2 + b2)                    # [B,K,128]
    out = h2 @ W3 + b3                         # [B,K]

Sharding: data-parallel over B across 8 NeuronCores (32 rows each);
weights replicated.  All activations kept transposed on-chip
([hidden, rows]) so the contraction dim sits on SBUF partitions and the
per-row k-tiles stream through the TensorEngine as the moving operand.
k is processed in 2 chunks of 500, zero-padded to 512 so every matmul
fills whole PSUM banks (no partially-written banks for the epilogues).

Device layout per core (host pre-packs everything contiguous):
  a2  [32,128,512]  action_feats[b].T k-chunks stacked on partitions
                    (p 0:64 = feats of k 0:500, p 64:128 = feats of k
                    500:1000; cols 500:512 zero pad)
  s2  [128,128]     state.T contraction chunks side by side (col c*32+b)
  w1s [128,4,256]   W1[:512] as 4 partition-chunks of 128
  w1a [128,256]     W1[512:] duplicated on partitions 0:64 / 64:128
  w2d [128,2,128]   W2 as 2 partition-chunks
  w3d [128,32]      W3 replicated to 32 cols (fills a 32-partition strip)
  b1d [128,2], b2d [128,1], b3d [128,1] (b3 broadcast to 128)
  out [32,1000]
"""

import os
import numpy as np

B, K = 256, 1000
SD, AD, H, G = 512, 64, 256, 128
NCORES = 8
BL = B // NCORES          # 32 batch rows per core
KC = 500                  # real k-chunk length (2 chunks per row)
KP = 512                  # padded k-chunk length (fills one PSUM bank)
GRP = 4                   # batch rows per input-DMA / L3 group
NCONST = 1700             # packed consts: w1s|s2|w1a|w2|w3|b1|b2|b3

_CACHE = {}
LAST_EXEC_NS = None


def _build_nc():
    from contextlib import ExitStack

    import concourse.bass as bass
    import concourse.bacc as bacc
    import concourse.mybir as mybir
    import concourse.tile as tile

    f32 = mybir.dt.float32
    f32r = mybir.dt.float32r
    AF = mybir.ActivationFunctionType
    ALU = mybir.AluOpType

    nc = bacc.Bacc("TRN2", target_bir_lowering=False, debug=False,
                   num_devices=NCORES)

    a2 = nc.dram_tensor("a2", [BL, 128, KP], f32r, kind="ExternalInput").ap()
    constsd = nc.dram_tensor("consts", [128, NCONST], f32r,
                             kind="ExternalInput").ap()
    out = nc.dram_tensor("out", [BL, K], f32, kind="ExternalOutput").ap()

    with tile.TileContext(nc) as tc, ExitStack() as ctx:
        wp = ctx.enter_context(tc.tile_pool(name="wp", bufs=1))
        xp = ctx.enter_context(tc.tile_pool(name="xp", bufs=2))
        h1p = ctx.enter_context(tc.tile_pool(name="h1p", bufs=3))
        h2p = ctx.enter_context(tc.tile_pool(name="h2p", bufs=GRP + 1))
        osp = ctx.enter_context(tc.tile_pool(name="osp", bufs=2))
        l1p = ctx.enter_context(tc.tile_pool(name="l1p", bufs=2, space="PSUM"))
        l2p = ctx.enter_context(tc.tile_pool(name="l2p", bufs=2, space="PSUM"))
        l3p = ctx.enter_context(tc.tile_pool(name="l3p", bufs=1, space="PSUM"))

        # ---- constants: single DMA so matmuls wait on one semaphore ----
        cs = wp.tile([128, NCONST], f32r)
        cs_dma = nc.sync.dma_start(cs[:], constsd[:])
        w1s_sb = cs[:, 0:1024].rearrange("p (c h) -> p c h", c=4)
        s2_sb = cs[:, 1024:1152]
        w1a_sb = cs[:, 1152:1408]
        w2_sb = cs[:, 1408:1664].rearrange("p (c g) -> p c g", c=2)
        w3_sb = cs[:, 1664:1696]
        b1_sb = cs[:, 1696:1698].bitcast(f32)
        b2_sb = cs[:, 1698:1699].bitcast(f32)
        b3_sb = cs[:, 1699:1700].bitcast(f32)

        # ---- epilogue engine balancer (ACT vs DVE) ----
        eng_ns = [0.0, 0.0]

        def ep(out_ap, in_ap, bias_ap, relu):
            fd = in_ap.free_size()
            cost_act = (172 + fd) / 1.2
            cost_dve = (58 + fd) / 0.96
            if eng_ns[0] + cost_act <= eng_ns[1] + cost_dve:
                eng_ns[0] += cost_act
                return nc.scalar.activation(out_ap, in_ap,
                                            AF.Relu if relu else AF.Identity,
                                            bias=bias_ap)
            eng_ns[1] += cost_dve
            if relu:
                return nc.vector.tensor_scalar(out_ap, in_ap, bias_ap, 0.0,
                                               ALU.add, ALU.max)
            return nc.vector.tensor_scalar(out_ap, in_ap, bias_ap, None,
                                           ALU.add)


        # ---- h_state = (state @ W1_s).T + b1 : [128, 2*BL], col h*BL+b ----
        hs_sb = wp.tile([128, 2 * BL], f32)
        for h in range(2):
            ps = l1p.tile([128, BL], f32, tag="l1")
            for c in range(4):
                nc.tensor.matmul(
                    ps[:],
                    lhsT=w1s_sb[:, c, 128 * h:128 * (h + 1)],
                    rhs=s2_sb[:, 32 * c:32 * (c + 1)],
                    start=(c == 0), stop=(c == 3))
            nc.scalar.activation(hs_sb[:, BL * h:BL * (h + 1)], ps[:],
                                 AF.Identity, bias=b1_sb[:, h:h + 1])

        # ---- main loop over groups of GRP batch rows ----
        for g in range(BL // GRP):
            xt = xp.tile([128, GRP, KP], f32r, tag="xt")
            xt_dma = nc.sync.dma_start(
                xt[:], a2[GRP * g:GRP * (g + 1), :, :].transpose([1, 0, 2]))

            osb = osp.tile([128, 2 * GRP * KP], f32, tag="osb")
            for j in range(GRP):
                b = GRP * g + j
                # L1: z1.T [256, 2*KP] in two partition chunks x two k-chunks
                h1t = []
                for h in range(2):
                    l1t = l1p.tile([128, 1024], f32, tag="l1")
                    for c in range(2):
                        nc.tensor.matmul(
                            l1t[:, KP * c:KP * (c + 1)],
                            lhsT=w1a_sb[64 * c:64 * (c + 1),
                                        128 * h:128 * (h + 1)],
                            rhs=xt[64 * c:64 * (c + 1), j, :],
                            start=True, stop=True)
                    h1 = h1p.tile([128, 1024], f32r, tag="h1")
                    ep(h1[:], l1t[:], hs_sb[:, BL * h + b:BL * h + b + 1],
                       relu=True)
                    h1t.append(h1)

                # L2: z2.T [128, 2*KP]
                h2 = h2p.tile([128, 1024], f32r, tag="h2")
                for c in range(2):
                    l2t = l2p.tile([128, 512], f32, tag="l2")
                    for hh in range(2):
                        nc.tensor.matmul(
                            l2t[:],
                            lhsT=w2_sb[:, hh, :],
                            rhs=h1t[hh][:, KP * c:KP * (c + 1)],
                            start=(hh == 0), stop=(hh == 1))
                    ep(h2[:, KP * c:KP * (c + 1)], l2t[:], b2_sb[:, 0:1],
                       relu=True)

                # L3: scores [1, KP] per k-chunk; bias-add lands them on
                # partition 0 of osb, one output DMA per batch row.
                for c in range(2):
                    l3t = l3p.tile([128, 512], f32, tag="l3")
                    nc.tensor.matmul(
                        l3t[0:1, :],
                        lhsT=w3_sb[:, 0:1],
                        rhs=h2[:, KP * c:KP * (c + 1)],
                        start=True, stop=True)
                    ep(osb[0:1, KP * (2 * j + c):KP * (2 * j + c + 1)],
                       l3t[0:1, :], b3_sb[0:1, 0:1], relu=False)
                nc.sync.dma_start(
                    out[b:b + 1, :].rearrange("b (c k) -> b c k", c=2),
                    osb[0:1, 2 * KP * j:2 * KP * (j + 1)].rearrange(
                        "p (c k) -> p c k", c=2)[:, :, 0:KC])

    nc.compile()
    return nc


def _prep_inputs(state_embed, action_feats, W1, b1, W2, b2, W3, b3):
    f4 = lambda x: np.ascontiguousarray(np.asarray(x, dtype=np.float32))
    state_embed, action_feats = f4(state_embed), f4(action_feats)
    W1, b1, W2, b2, W3, b3 = map(f4, (W1, b1, W2, b2, W3, b3))

    W1s, W1a = W1[:SD], W1[SD:]
    w1s_h = np.concatenate([W1s[c * 128:(c + 1) * 128] for c in range(4)],
                           axis=1)                           # [128, 1024]
    w1a_h = np.concatenate([W1a, W1a], axis=0)               # [128, 256]
    w2_h = np.concatenate([W2[:128], W2[128:]], axis=1)      # [128, 256]
    w3_h = np.repeat(W3.reshape(G, 1), 32, axis=1)           # [128, 32]
    b1_h = b1.reshape(2, 128).T                              # [128, 2]
    b2_h = b2.reshape(G, 1)
    b3_h = np.broadcast_to(b3.reshape(1), (128,)).reshape(128, 1)

    in_maps = []
    for ci in range(NCORES):
        sl = slice(ci * BL, (ci + 1) * BL)
        aft = action_feats[sl].transpose(0, 2, 1)            # [BL, 64, 1000]
        a2_h = np.zeros((BL, 128, KP), dtype=np.float32)
        a2_h[:, 0:64, 0:KC] = aft[:, :, :KC]
        a2_h[:, 64:128, 0:KC] = aft[:, :, KC:]
        st = state_embed[sl].T                               # [512, BL]
        s2_h = np.concatenate([st[c * 128:(c + 1) * 128] for c in range(4)],
                              axis=1)                        # [128, 128]
        consts = np.ascontiguousarray(np.concatenate(
            [w1s_h, s2_h, w1a_h, w2_h, w3_h, b1_h, b2_h, b3_h], axis=1))
        assert consts.shape == (128, NCONST), consts.shape
        in_maps.append({"a2": a2_h, "consts": consts})
    return in_maps


def kernel(state_embed, action_feats, W1, b1, W2, b2, W3, b3):
    global LAST_EXEC_NS
    from concourse.bass_utils import run_bass_kernel_spmd

    if "nc" not in _CACHE:
        _CACHE["nc"] = _build_nc()
    nc = _CACHE["nc"]

    in_maps = _prep_inputs(state_embed, action_feats, W1, b1, W2, b2, W3, b3)
    trace = bool(int(os.environ.get("ACTOR_KERNEL_TRACE", "0")))
    res = run_bass_kernel_spmd(nc, in_maps, core_ids=list(range(NCORES)),
                               trace=trace)
    LAST_EXEC_NS = res.exec_time_ns
    outs = [np.asarray(res.results[i]["out"]) for i in range(NCORES)]
    return np.ascontiguousarray(np.concatenate(outs, axis=0).astype(np.float32))


# revision 22
# speedup vs baseline: 1.0518x; 1.0518x over previous
"""Trainium2 Bass kernel: 3-layer actor MLP over [B=256, K=1000] actions.

Math (per reference):
    h1 = relu(af @ W1_a + state @ W1_s + b1)   # [B,K,256]
    h2 = relu(h1 @ W2 + b2)                    # [B,K,128]
    out = h2 @ W3 + b3                         # [B,K]

Sharding: data-parallel over B across 8 NeuronCores (32 rows each);
weights replicated.  All activations kept transposed on-chip
([hidden, rows]) so the contraction dim sits on SBUF partitions and the
per-row k-tiles stream through the TensorEngine as the moving operand.
k is processed in 2 chunks of 500, zero-padded to 512 so every matmul
fills whole PSUM banks (no partially-written banks for the epilogues).

Device layout per core (host pre-packs everything contiguous):
  a2  [32,128,512]  action_feats[b].T k-chunks stacked on partitions
                    (p 0:64 = feats of k 0:500, p 64:128 = feats of k
                    500:1000; cols 500:512 zero pad)
  s2  [128,128]     state.T contraction chunks side by side (col c*32+b)
  w1s [128,4,256]   W1[:512] as 4 partition-chunks of 128
  w1a [128,256]     W1[512:] duplicated on partitions 0:64 / 64:128
  w2d [128,2,128]   W2 as 2 partition-chunks
  w3d [128,32]      W3 replicated to 32 cols (fills a 32-partition strip)
  b1d [128,2], b2d [128,1], b3d [128,1] (b3 broadcast to 128)
  out [32,1000]
"""

import os
import numpy as np

B, K = 256, 1000
SD, AD, H, G = 512, 64, 256, 128
NCORES = 8
BL = B // NCORES          # 32 batch rows per core
KC = 500                  # real k-chunk length (2 chunks per row)
KP = 512                  # padded k-chunk length (fills one PSUM bank)
GRP = 4                   # batch rows per input-DMA / L3 group
NCONST = 1700             # packed consts: w1s|s2|w1a|w2|w3|b1|b2|b3

_CACHE = {}
LAST_EXEC_NS = None


def _build_nc():
    from contextlib import ExitStack

    import concourse.bass as bass
    import concourse.bacc as bacc
    import concourse.mybir as mybir
    import concourse.tile as tile

    f32 = mybir.dt.float32
    f32r = mybir.dt.float32r
    AF = mybir.ActivationFunctionType
    ALU = mybir.AluOpType

    nc = bacc.Bacc("TRN2", target_bir_lowering=False, debug=False,
                   num_devices=NCORES)

    a2 = nc.dram_tensor("a2", [BL, 128, KP], f32r, kind="ExternalInput").ap()
    constsd = nc.dram_tensor("consts", [128, NCONST], f32r,
                             kind="ExternalInput").ap()
    out = nc.dram_tensor("out", [BL, K], f32, kind="ExternalOutput").ap()

    with tile.TileContext(nc) as tc, ExitStack() as ctx:
        wp = ctx.enter_context(tc.tile_pool(name="wp", bufs=1))
        xp = ctx.enter_context(tc.tile_pool(name="xp", bufs=2))
        h1p = ctx.enter_context(tc.tile_pool(name="h1p", bufs=3))
        h2p = ctx.enter_context(tc.tile_pool(name="h2p", bufs=GRP + 1))
        osp = ctx.enter_context(tc.tile_pool(name="osp", bufs=2))
        l1p = ctx.enter_context(tc.tile_pool(name="l1p", bufs=2, space="PSUM"))
        l2p = ctx.enter_context(tc.tile_pool(name="l2p", bufs=2, space="PSUM"))
        l3p = ctx.enter_context(tc.tile_pool(name="l3p", bufs=1, space="PSUM"))

        # ---- constants: single DMA so matmuls wait on one semaphore ----
        cs = wp.tile([128, NCONST], f32r)
        cs_dma = nc.sync.dma_start(cs[:], constsd[:])
        w1s_sb = cs[:, 0:1024].rearrange("p (c h) -> p c h", c=4)
        s2_sb = cs[:, 1024:1152]
        w1a_sb = cs[:, 1152:1408]
        w2_sb = cs[:, 1408:1664].rearrange("p (c g) -> p c g", c=2)
        w3_sb = cs[:, 1664:1696]
        b1_sb = cs[:, 1696:1698].bitcast(f32)
        b2_sb = cs[:, 1698:1699].bitcast(f32)
        b3_sb = cs[:, 1699:1700].bitcast(f32)

        # ---- epilogue engine balancer (ACT vs DVE) ----
        eng_ns = [0.0, 0.0]

        def ep(out_ap, in_ap, bias_ap, relu):
            fd = in_ap.free_size()
            cost_act = (172 + fd) / 1.2
            cost_dve = (58 + fd) / 0.96
            if eng_ns[0] + cost_act <= eng_ns[1] + cost_dve:
                eng_ns[0] += cost_act
                return nc.scalar.activation(out_ap, in_ap,
                                            AF.Relu if relu else AF.Identity,
                                            bias=bias_ap)
            eng_ns[1] += cost_dve
            if relu:
                return nc.vector.tensor_scalar(out_ap, in_ap, bias_ap, 0.0,
                                               ALU.add, ALU.max)
            return nc.vector.tensor_scalar(out_ap, in_ap, bias_ap, None,
                                           ALU.add)


        # ---- PE warm-up: ~8us of dummy matmuls while the first DMAs land,
        # so the HAM clock-gate releases (1.2 -> 2.4 GHz) before real work.
        bf16 = mybir.dt.bfloat16
        dummy = wp.tile([64, 576], bf16)
        nc.vector.memset(dummy[:], 0.0)
        wps = l3p.tile([128, 512], f32, tag="l3")
        for _ in range(20):
            nc.tensor.matmul(wps[0:64, :], lhsT=dummy[:, 512:576],
                             rhs=dummy[:, 0:512], start=True, stop=True)

        # ---- h_state = (state @ W1_s).T + b1 : [128, 2*BL], col h*BL+b ----
        hs_sb = wp.tile([128, 2 * BL], f32)
        for h in range(2):
            ps = l1p.tile([128, BL], f32, tag="l1")
            for c in range(4):
                nc.tensor.matmul(
                    ps[:],
                    lhsT=w1s_sb[:, c, 128 * h:128 * (h + 1)],
                    rhs=s2_sb[:, 32 * c:32 * (c + 1)],
                    start=(c == 0), stop=(c == 3))
            nc.scalar.activation(hs_sb[:, BL * h:BL * (h + 1)], ps[:],
                                 AF.Identity, bias=b1_sb[:, h:h + 1])

        # ---- main loop over groups of GRP batch rows ----
        for g in range(BL // GRP):
            xt = xp.tile([128, GRP, KP], f32r, tag="xt")
            xt_dma = nc.sync.dma_start(
                xt[:], a2[GRP * g:GRP * (g + 1), :, :].transpose([1, 0, 2]))

            osb = osp.tile([128, 2 * GRP * KP], f32, tag="osb")
            for j in range(GRP):
                b = GRP * g + j
                # L1: z1.T [256, 2*KP] in two partition chunks x two k-chunks
                h1t = []
                for h in range(2):
                    l1t = l1p.tile([128, 1024], f32, tag="l1")
                    for c in range(2):
                        nc.tensor.matmul(
                            l1t[:, KP * c:KP * (c + 1)],
                            lhsT=w1a_sb[64 * c:64 * (c + 1),
                                        128 * h:128 * (h + 1)],
                            rhs=xt[64 * c:64 * (c + 1), j, :],
                            start=True, stop=True)
                    h1 = h1p.tile([128, 1024], f32r, tag="h1")
                    ep(h1[:], l1t[:], hs_sb[:, BL * h + b:BL * h + b + 1],
                       relu=True)
                    h1t.append(h1)

                # L2: z2.T [128, 2*KP]
                h2 = h2p.tile([128, 1024], f32r, tag="h2")
                for c in range(2):
                    l2t = l2p.tile([128, 512], f32, tag="l2")
                    for hh in range(2):
                        nc.tensor.matmul(
                            l2t[:],
                            lhsT=w2_sb[:, hh, :],
                            rhs=h1t[hh][:, KP * c:KP * (c + 1)],
                            start=(hh == 0), stop=(hh == 1))
                    ep(h2[:, KP * c:KP * (c + 1)], l2t[:], b2_sb[:, 0:1],
                       relu=True)

                # L3: scores [1, KP] per k-chunk; bias-add lands them on
                # partition 0 of osb, one output DMA per batch row.
                for c in range(2):
                    l3t = l3p.tile([128, 512], f32, tag="l3")
                    nc.tensor.matmul(
                        l3t[0:1, :],
                        lhsT=w3_sb[:, 0:1],
                        rhs=h2[:, KP * c:KP * (c + 1)],
                        start=True, stop=True)
                    ep(osb[0:1, KP * (2 * j + c):KP * (2 * j + c + 1)],
                       l3t[0:1, :], b3_sb[0:1, 0:1], relu=False)
                nc.sync.dma_start(
                    out[b:b + 1, :].rearrange("b (c k) -> b c k", c=2),
                    osb[0:1, 2 * KP * j:2 * KP * (j + 1)].rearrange(
                        "p (c k) -> p c k", c=2)[:, :, 0:KC])

    nc.compile()
    return nc


def _prep_inputs(state_embed, action_feats, W1, b1, W2, b2, W3, b3):
    f4 = lambda x: np.ascontiguousarray(np.asarray(x, dtype=np.float32))
    state_embed, action_feats = f4(state_embed), f4(action_feats)
    W1, b1, W2, b2, W3, b3 = map(f4, (W1, b1, W2, b2, W3, b3))

    W1s, W1a = W1[:SD], W1[SD:]
    w1s_h = np.concatenate([W1s[c * 128:(c + 1) * 128] for c in range(4)],
                           axis=1)                           # [128, 1024]
    w1a_h = np.concatenate([W1a, W1a], axis=0)               # [128, 256]
    w2_h = np.concatenate([W2[:128], W2[128:]], axis=1)      # [128, 256]
    w3_h = np.repeat(W3.reshape(G, 1), 32, axis=1)           # [128, 32]
    b1_h = b1.reshape(2, 128).T                              # [128, 2]
    b2_h = b2.reshape(G, 1)
    b3_h = np.broadcast_to(b3.reshape(1), (128,)).reshape(128, 1)

    in_maps = []
    for ci in range(NCORES):
        sl = slice(ci * BL, (ci + 1) * BL)
        aft = action_feats[sl].transpose(0, 2, 1)            # [BL, 64, 1000]
        a2_h = np.zeros((BL, 128, KP), dtype=np.float32)
        a2_h[:, 0:64, 0:KC] = aft[:, :, :KC]
        a2_h[:, 64:128, 0:KC] = aft[:, :, KC:]
        st = state_embed[sl].T                               # [512, BL]
        s2_h = np.concatenate([st[c * 128:(c + 1) * 128] for c in range(4)],
                              axis=1)                        # [128, 128]
        consts = np.ascontiguousarray(np.concatenate(
            [w1s_h, s2_h, w1a_h, w2_h, w3_h, b1_h, b2_h, b3_h], axis=1))
        assert consts.shape == (128, NCONST), consts.shape
        in_maps.append({"a2": a2_h, "consts": consts})
    return in_maps


def kernel(state_embed, action_feats, W1, b1, W2, b2, W3, b3):
    global LAST_EXEC_NS
    from concourse.bass_utils import run_bass_kernel_spmd

    if "nc" not in _CACHE:
        _CACHE["nc"] = _build_nc()
    nc = _CACHE["nc"]

    in_maps = _prep_inputs(state_embed, action_feats, W1, b1, W2, b2, W3, b3)
    trace = bool(int(os.environ.get("ACTOR_KERNEL_TRACE", "0")))
    res = run_bass_kernel_spmd(nc, in_maps, core_ids=list(range(NCORES)),
                               trace=trace)
    LAST_EXEC_NS = res.exec_time_ns
    outs = [np.asarray(res.results[i]["out"]) for i in range(NCORES)]
    return np.ascontiguousarray(np.concatenate(outs, axis=0).astype(np.float32))


# revision 24
# speedup vs baseline: 1.1399x; 1.0838x over previous
"""Trainium2 Bass kernel: 3-layer actor MLP over [B=256, K=1000] actions.

Math (per reference):
    h1 = relu(af @ W1_a + state @ W1_s + b1)   # [B,K,256]
    h2 = relu(h1 @ W2 + b2)                    # [B,K,128]
    out = h2 @ W3 + b3                         # [B,K]

Sharding: data-parallel over B across 8 NeuronCores (32 rows each);
weights replicated.  All activations kept transposed on-chip
([hidden, rows]) so the contraction dim sits on SBUF partitions and the
per-row k-tiles stream through the TensorEngine as the moving operand.
k is processed in 2 chunks of 500, zero-padded to 512 so every matmul
fills whole PSUM banks (no partially-written banks for the epilogues).

Device layout per core (host pre-packs everything contiguous):
  a2  [32,128,512]  action_feats[b].T k-chunks stacked on partitions
                    (p 0:64 = feats of k 0:500, p 64:128 = feats of k
                    500:1000; cols 500:512 zero pad)
  s2  [128,128]     state.T contraction chunks side by side (col c*32+b)
  w1s [128,4,256]   W1[:512] as 4 partition-chunks of 128
  w1a [128,256]     W1[512:] duplicated on partitions 0:64 / 64:128
  w2d [128,2,128]   W2 as 2 partition-chunks
  w3d [128,32]      W3 replicated to 32 cols (fills a 32-partition strip)
  b1d [128,2], b2d [128,1], b3d [128,1] (b3 broadcast to 128)
  out [32,1000]
"""

import os
import numpy as np

B, K = 256, 1000
SD, AD, H, G = 512, 64, 256, 128
NCORES = 8
BL = B // NCORES          # 32 batch rows per core
KC = 500                  # real k-chunk length (2 chunks per row)
KP = 512                  # padded k-chunk length (fills one PSUM bank)
GRP = 4                   # batch rows per input-DMA / L3 group
NCONST = 1696             # bf16 cols: w1s|s2|w1a|w2|w3

_CACHE = {}
LAST_EXEC_NS = None


def _build_nc():
    from contextlib import ExitStack

    import concourse.bass as bass
    import concourse.bacc as bacc
    import concourse.mybir as mybir
    import concourse.tile as tile

    f32 = mybir.dt.float32
    bf16 = mybir.dt.bfloat16
    AF = mybir.ActivationFunctionType
    ALU = mybir.AluOpType

    nc = bacc.Bacc("TRN2", target_bir_lowering=False, debug=False,
                   num_devices=NCORES)

    a2 = nc.dram_tensor("a2", [BL, 128, KP], bf16, kind="ExternalInput").ap()
    constsd = nc.dram_tensor("consts", [128, NCONST], bf16,
                             kind="ExternalInput").ap()
    biasd = nc.dram_tensor("biases", [128, 4], f32, kind="ExternalInput").ap()
    out = nc.dram_tensor("out", [BL, K], f32, kind="ExternalOutput").ap()

    with tile.TileContext(nc) as tc, ExitStack() as ctx:
        wp = ctx.enter_context(tc.tile_pool(name="wp", bufs=1))
        xp = ctx.enter_context(tc.tile_pool(name="xp", bufs=2))
        h1p = ctx.enter_context(tc.tile_pool(name="h1p", bufs=3))
        h2p = ctx.enter_context(tc.tile_pool(name="h2p", bufs=GRP + 1))
        osp = ctx.enter_context(tc.tile_pool(name="osp", bufs=2))
        l1p = ctx.enter_context(tc.tile_pool(name="l1p", bufs=2, space="PSUM"))
        l2p = ctx.enter_context(tc.tile_pool(name="l2p", bufs=2, space="PSUM"))
        l3p = ctx.enter_context(tc.tile_pool(name="l3p", bufs=1, space="PSUM"))

        # ---- constants: single DMA so matmuls wait on one semaphore ----
        cs = wp.tile([128, NCONST], bf16)
        cs_dma = nc.sync.dma_start(cs[:], constsd[:])
        w1s_sb = cs[:, 0:1024].rearrange("p (c h) -> p c h", c=4)
        s2_sb = cs[:, 1024:1152]
        w1a_sb = cs[:, 1152:1408]
        w2_sb = cs[:, 1408:1664].rearrange("p (c g) -> p c g", c=2)
        w3_sb = cs[:, 1664:1696]
        bb = wp.tile([128, 4], f32)
        nc.sync.dma_start(bb[:], biasd[:])
        b1_sb = bb[:, 0:2]
        b2_sb = bb[:, 2:3]
        b3_sb = bb[:, 3:4]

        # ---- epilogue engine balancer (ACT vs DVE) ----
        eng_ns = [0.0, 0.0]

        def ep(out_ap, in_ap, bias_ap, relu):
            fd = in_ap.free_size()
            cost_act = (172 + fd) / 1.2
            cost_dve = (58 + fd) / 0.96
            if eng_ns[0] + cost_act <= eng_ns[1] + cost_dve:
                eng_ns[0] += cost_act
                return nc.scalar.activation(out_ap, in_ap,
                                            AF.Relu if relu else AF.Identity,
                                            bias=bias_ap)
            eng_ns[1] += cost_dve
            if relu:
                return nc.vector.tensor_scalar(out_ap, in_ap, bias_ap, 0.0,
                                               ALU.add, ALU.max)
            return nc.vector.tensor_scalar(out_ap, in_ap, bias_ap, None,
                                           ALU.add)


        # ---- PE warm-up: ~8us of dummy matmuls while the first DMAs land,
        # so the HAM clock-gate releases (1.2 -> 2.4 GHz) before real work.
        dummy = wp.tile([64, 576], bf16)
        nc.vector.memset(dummy[:], 0.0)
        wps = l3p.tile([128, 512], f32, tag="l3")
        for _ in range(20):
            nc.tensor.matmul(wps[0:64, :], lhsT=dummy[:, 512:576],
                             rhs=dummy[:, 0:512], start=True, stop=True)

        # ---- h_state = (state @ W1_s).T + b1 : [128, 2*BL], col h*BL+b ----
        hs_sb = wp.tile([128, 2 * BL], f32)
        for h in range(2):
            ps = l1p.tile([128, BL], f32, tag="l1")
            for c in range(4):
                nc.tensor.matmul(
                    ps[:],
                    lhsT=w1s_sb[:, c, 128 * h:128 * (h + 1)],
                    rhs=s2_sb[:, 32 * c:32 * (c + 1)],
                    start=(c == 0), stop=(c == 3))
            nc.scalar.activation(hs_sb[:, BL * h:BL * (h + 1)], ps[:],
                                 AF.Identity, bias=b1_sb[:, h:h + 1])

        # ---- main loop over groups of GRP batch rows ----
        for g in range(BL // GRP):
            xt = xp.tile([128, GRP, KP], bf16, tag="xt")
            xt_dma = nc.sync.dma_start(
                xt[:], a2[GRP * g:GRP * (g + 1), :, :].transpose([1, 0, 2]))

            osb = osp.tile([128, 2 * GRP * KP], f32, tag="osb")
            for j in range(GRP):
                b = GRP * g + j
                # L1: z1.T [256, 2*KP] in two partition chunks x two k-chunks
                h1t = []
                for h in range(2):
                    l1t = l1p.tile([128, 1024], f32, tag="l1")
                    for c in range(2):
                        nc.tensor.matmul(
                            l1t[:, KP * c:KP * (c + 1)],
                            lhsT=w1a_sb[64 * c:64 * (c + 1),
                                        128 * h:128 * (h + 1)],
                            rhs=xt[64 * c:64 * (c + 1), j, :],
                            start=True, stop=True)
                    h1 = h1p.tile([128, 1024], bf16, tag="h1")
                    ep(h1[:], l1t[:], hs_sb[:, BL * h + b:BL * h + b + 1],
                       relu=True)
                    h1t.append(h1)

                # L2: z2.T [128, 2*KP]
                h2 = h2p.tile([128, 1024], bf16, tag="h2")
                for c in range(2):
                    l2t = l2p.tile([128, 512], f32, tag="l2")
                    for hh in range(2):
                        nc.tensor.matmul(
                            l2t[:],
                            lhsT=w2_sb[:, hh, :],
                            rhs=h1t[hh][:, KP * c:KP * (c + 1)],
                            start=(hh == 0), stop=(hh == 1))
                    ep(h2[:, KP * c:KP * (c + 1)], l2t[:], b2_sb[:, 0:1],
                       relu=True)

                # L3: scores [1, KP] per k-chunk; bias-add lands them on
                # partition 0 of osb, one output DMA per batch row.
                for c in range(2):
                    l3t = l3p.tile([128, 512], f32, tag="l3")
                    nc.tensor.matmul(
                        l3t[0:1, :],
                        lhsT=w3_sb[:, 0:1],
                        rhs=h2[:, KP * c:KP * (c + 1)],
                        start=True, stop=True)
                    ep(osb[0:1, KP * (2 * j + c):KP * (2 * j + c + 1)],
                       l3t[0:1, :], b3_sb[0:1, 0:1], relu=False)
                nc.sync.dma_start(
                    out[b:b + 1, :].rearrange("b (c k) -> b c k", c=2),
                    osb[0:1, 2 * KP * j:2 * KP * (j + 1)].rearrange(
                        "p (c k) -> p c k", c=2)[:, :, 0:KC])

    nc.compile()
    return nc


def _prep_inputs(state_embed, action_feats, W1, b1, W2, b2, W3, b3):
    import ml_dtypes
    bf = ml_dtypes.bfloat16
    f4 = lambda x: np.ascontiguousarray(np.asarray(x, dtype=np.float32))
    state_embed, action_feats = f4(state_embed), f4(action_feats)
    W1, b1, W2, b2, W3, b3 = map(f4, (W1, b1, W2, b2, W3, b3))

    W1s, W1a = W1[:SD], W1[SD:]
    w1s_h = np.concatenate([W1s[c * 128:(c + 1) * 128] for c in range(4)],
                           axis=1).astype(bf)                # [128, 1024]
    w1a_h = np.concatenate([W1a, W1a], axis=0).astype(bf)    # [128, 256]
    w2_h = np.concatenate([W2[:128], W2[128:]], axis=1).astype(bf)
    w3_h = np.repeat(W3.reshape(G, 1), 32, axis=1).astype(bf)
    biases = np.ascontiguousarray(np.concatenate(
        [b1.reshape(2, 128).T, b2.reshape(G, 1),
         np.broadcast_to(b3.reshape(1), (128,)).reshape(128, 1)],
        axis=1))                                              # [128, 4] f32

    in_maps = []
    for ci in range(NCORES):
        sl = slice(ci * BL, (ci + 1) * BL)
        aft = action_feats[sl].transpose(0, 2, 1)            # [BL, 64, 1000]
        a2_h = np.zeros((BL, 128, KP), dtype=bf)
        a2_h[:, 0:64, 0:KC] = aft[:, :, :KC].astype(bf)
        a2_h[:, 64:128, 0:KC] = aft[:, :, KC:].astype(bf)
        st = state_embed[sl].T.astype(bf)                    # [512, BL]
        s2_h = np.concatenate([st[c * 128:(c + 1) * 128] for c in range(4)],
                              axis=1)                        # [128, 128]
        consts = np.ascontiguousarray(np.concatenate(
            [w1s_h, s2_h, w1a_h, w2_h, w3_h], axis=1))
        assert consts.shape == (128, NCONST), consts.shape
        in_maps.append({"a2": a2_h, "consts": consts, "biases": biases})
    return in_maps


def kernel(state_embed, action_feats, W1, b1, W2, b2, W3, b3):
    global LAST_EXEC_NS
    from concourse.bass_utils import run_bass_kernel_spmd

    if "nc" not in _CACHE:
        _CACHE["nc"] = _build_nc()
    nc = _CACHE["nc"]

    in_maps = _prep_inputs(state_embed, action_feats, W1, b1, W2, b2, W3, b3)
    trace = bool(int(os.environ.get("ACTOR_KERNEL_TRACE", "0")))
    res = run_bass_kernel_spmd(nc, in_maps, core_ids=list(range(NCORES)),
                               trace=trace)
    LAST_EXEC_NS = res.exec_time_ns
    outs = [np.asarray(res.results[i]["out"]) for i in range(NCORES)]
    return np.ascontiguousarray(np.concatenate(outs, axis=0).astype(np.float32))
